# revision 43
# baseline (speedup 1.0000x reference)
"""Trainium2 Bass kernel for LocalGraphProjection (perceptual feature pooling).

Pipeline per point: project through 3 cameras, bilinear-sample 3 feature
pyramid levels per view (16/32/64 ch), concat -> [N,112] per view, then
max/mean/std across views -> [N, 3+336] output.

Strategy:
  - Host folds cameras into one affine per view (fp64 -> fp32 consts).
  - Host rebuilds each pyramid level as a table of 2x2 clamped-neighbor
    patches in fp16, so one bilinear sample = ONE dma_gather descriptor.
  - 8 cores data-parallel over points. Per core: points live on
    [128 partitions x 256 slots]; per-point math in fp32; gathers via
    GPSIMD SWDGE dma_gather; weighted combine + view stats in fp16.
  - Device returns [128, 256, 336] fp16 per core; host prepends coord and
    upcasts to fp32.
"""

import numpy as np

import concourse.bass as bass
import concourse.bacc as bacc
import concourse.mybir as mybir
from concourse.tile import TileContext
from concourse.bass_utils import run_bass_kernel_spmd
from concourse import library_config

F32 = mybir.dt.float32
F16 = mybir.dt.float16
I16 = mybir.dt.int16
I32 = mybir.dt.int32
ALU = mybir.AluOpType
ACTF = mybir.ActivationFunctionType

PI = float(np.pi)

N_PTS = 262144
N_CORES = 8
N_CORE_PTS = N_PTS // N_CORES  # 32768
P = 128
M = N_CORE_PTS // P  # 256 slots per partition

# points per gather tile
T = 1024
MT = T // P  # 8
NT = M // MT  # 32

# level geometry: (H, W, C, S) with S = corners per patch entry
LEV = [
    (224, 224, 16, 8),   # L0: entry = 2 rows x 4 cols (col-pair aligned)
    (112, 112, 32, 4),   # L1: entry = 2x2
    (56, 56, 64, 4),     # L2: entry = 2x2
]
SCALES = [1.0, 2.0, 4.0]
# merged table: one entry per (x1, y1-pair) key carrying all 3 levels
TAB_ROWS = 224 * 112
ENT_OFF = [0, 128, 256]      # level slice starts within an entry
ENT_ALL = 512                # fp16 elements per entry (1024 B)


# ----------------------------------------------------------------- host math
def _camera_affines(cameras: np.ndarray):
    """Per-view affine pc = coord @ A_v + b_v, in float64 (mirrors reference)."""
    cams = cameras.astype(np.float64)

    def cm(param):
        theta = param[0] * (PI / 180.0)
        camy = param[3] * np.sin(param[1] * PI / 180.0)
        lens = param[3] * np.cos(param[1] * PI / 180.0)
        camx = lens * np.cos(theta)
        camz = lens * np.sin(theta)
        Zv = np.array([camx, camy, camz])
        Yv = np.array([camy * np.cos(theta + PI), lens, camy * np.sin(theta + PI)])
        Xv = np.cross(Yv, Zv)
        c = np.stack(
            [Xv / np.linalg.norm(Xv), Yv / np.linalg.norm(Yv), Zv / np.linalg.norm(Zv)]
        )
        return c, Zv

    c0, o0 = cm(cams[0])
    M0 = np.linalg.inv(c0.T)
    A, B = [], []
    for v in range(3):
        cv, ov = cm(cams[v])
        A.append(M0 @ cv.T)            # [3,3]
        B.append((o0 - ov) @ cv.T)     # [3]
    return A, B


def _build_affine_plane(cameras: np.ndarray) -> np.ndarray:
    """[P, 40] fp32: per view v, 12 consts at col v*13:
    [a00,a10,a20,b0, -a01,-a11,-a21,-b1, -a02,-a12,-a22,-b2]."""
    A, B = _camera_affines(cameras)
    row = np.zeros(40, np.float32)
    for v in range(3):
        a, b = A[v], B[v]
        base = v * 13
        row[base + 0 : base + 3] = a[:, 0]
        row[base + 3] = b[0]
        row[base + 4 : base + 7] = -a[:, 1]
        row[base + 7] = -b[1]
        row[base + 8 : base + 11] = -a[:, 2]
        row[base + 11] = -b[2]
    return np.tile(row[None, :], (P, 1))


def _build_tables(img_feat0, img_feat1, img_feat2):
    """Merged pyramid table per view, fp16, slot-major entries [slots, C].

    One entry per L0 cell key (x1, jp) with jp = y1//2 (col-pair aligned):
      [0:128)   L0: rows {x1, x1+1c} x cols {2jp..2jp+3c}      [2, 4, 16]
      [128:256) L1: rows {x1//2, +1c} x cols {jp, jp+1c}       [2, 2, 32]
      [256:512) L2: rows {x1//4, +1c} x cols {jp//2, +1c}      [2, 2, 64]
    The coarser windows are exact functions of the entry key, so one
    gather per (point, view) serves all three bilinear samples.
    """
    tabs = {}
    feats = [np.asarray(img_feat0), np.asarray(img_feat1), np.asarray(img_feat2)]
    H, W = 224, 112  # entry grid: x1 in [0,224), jp in [0,112)
    x1 = np.arange(224)
    jp = np.arange(112)
    for v in range(3):
        F0 = feats[0][v]  # [224,224,16]
        rows0 = np.stack([x1, np.minimum(x1 + 1, 223)], 1)                 # [224,2]
        cols0 = np.minimum(2 * jp[:, None] + np.arange(4)[None, :], 223)   # [112,4]
        t0 = F0[rows0][:, :, cols0]                      # [224, 2, 112, 4, 16]
        p0 = np.transpose(t0, (0, 2, 1, 3, 4)).reshape(H * W, 128)

        F1 = feats[1][v]  # [112,112,32]
        i1 = x1 // 2
        rows1 = np.stack([i1, np.minimum(i1 + 1, 111)], 1)                 # [224,2]
        cols1 = np.stack([jp, np.minimum(jp + 1, 111)], 1)                 # [112,2]
        t1 = F1[rows1][:, :, cols1]                      # [224, 2, 112, 2, 32]
        p1 = np.transpose(t1, (0, 2, 1, 3, 4)).reshape(H * W, 128)

        F2 = feats[2][v]  # [56,56,64]
        i2 = x1 // 4
        j2 = jp // 2
        rows2 = np.stack([i2, np.minimum(i2 + 1, 55)], 1)                  # [224,2]
        cols2 = np.stack([j2, np.minimum(j2 + 1, 55)], 1)                  # [112,2]
        t2 = F2[rows2][:, :, cols2]                      # [224, 2, 112, 2, 64]
        p2 = np.transpose(t2, (0, 2, 1, 3, 4)).reshape(H * W, 256)

        tabs[v] = np.ascontiguousarray(
            np.concatenate([p0, p1, p2], axis=1).astype(np.float16)
        )
    return tabs


# ------------------------------------------------------------- device kernel
def emit_body(nc, tc, pools, dram, m_total=M, mt=MT):
    """Emit the whole per-core program inside an open TileContext."""
    nt = m_total // mt
    sc, wpool, gpool, fpool, opool, ipool = (
        pools["sc"], pools["w"], pools["g"], pools["f"], pools["o"], pools["i"],
    )
    coords_d, afp_d, tabs_d, out_d = (
        dram["coords"], dram["afp"], dram["tabs"], dram["out"],
    )

    V = nc.vector
    G = nc.gpsimd
    S = nc.scalar
    IO = nc.sync

    import os as _os
    SKIP_GATHER = _os.environ.get("LGP_SKIP_GATHER", "0") == "1"
    SKIP_RECIP = _os.environ.get("LGP_SKIP_RECIP", "0") == "1"
    G.load_library(library_config.mlp)
    nidx_reg = G.alloc_register("nidx")
    G.reg_mov(nidx_reg, P * mt)

    # ---- preload
    coords_sb = sc.tile([P, 3, m_total], F32, tag="coords", name="coords_sb")
    IO.dma_start(out=coords_sb[:, :, :], in_=coords_d[:, :, :].transpose([1, 0, 2]))
    afp = sc.tile([P, 40], F32, tag="afp", name="afp_sb")
    IO.dma_start(out=afp[:, :], in_=afp_d[:, :])

    def ap_s(col):  # [P,1] scalar AP
        return afp[:, col : col + 1]

    cx = coords_sb[:, 0, :]
    cy = coords_sb[:, 1, :]
    cz = coords_sb[:, 2, :]

    # ---- whole-core per-point math (fp32, [P, m_total])
    # per view: one pair-duplicated weight tile covering all 3 levels
    # ([0:16) L0 slots, [16:24) L1, [24:32) L2) + one gather index.
    w_tiles = {}   # v -> [P, m_total, 32] fp16
    # indices written directly as int16 (values are exact small ints)
    idx_i = ipool.tile([P, 3, m_total], I16, tag="idxi", name="idxi", bufs=1)
    WOFF = [0, 8, 12]  # weight slot offsets per level (x2 for dup elements)

    def newt(tag, dt=F32, d3=None, pool=sc):
        shape = [P, m_total] if d3 is None else [P, m_total, d3]
        return pool.tile(shape, dt, tag=tag, name=tag)

    def dup2(src):
        return src[:, :].unsqueeze(2).broadcast_to([P, m_total, 2])

    def floor_pair(x, x1, fx, xi):
        # floor via int round-trip + compare fixup (x >= 0; no mod in ISA)
        V.tensor_copy(xi[:, :], x[:, :])
        V.tensor_copy(x1[:, :], xi[:, :])
        V.tensor_tensor(fx[:, :], x1[:, :], x[:, :], ALU.is_gt)
        V.tensor_tensor(x1[:, :], x1[:, :], fx[:, :], ALU.subtract)
        V.tensor_tensor(fx[:, :], x[:, :], x1[:, :], ALU.subtract)

    # ---- pass 1: projection + gather indices for all views (lets the
    # gather pipeline start while pass 2 computes the combine weights)
    HV, WV, FX0, FY0, PV = {}, {}, {}, {}, {}
    for v in range(3):
        base = v * 13
        X = newt("Xs")
        nY = newt("nYs")
        nZ = newt("nZs")
        for out_t, off in ((X, 0), (nY, 4), (nZ, 8)):
            V.tensor_scalar(out_t[:, :], cx, ap_s(base + off + 0), None, ALU.mult)
            V.scalar_tensor_tensor(
                out_t[:, :], cy, ap_s(base + off + 1), out_t[:, :], ALU.mult, ALU.add
            )
            V.scalar_tensor_tensor(
                out_t[:, :], cz, ap_s(base + off + 2), out_t[:, :], ALU.mult, ALU.add
            )
            V.tensor_scalar(out_t[:, :], out_t[:, :], ap_s(base + off + 3), None, ALU.add)
        rz = newt("rzs")
        if SKIP_RECIP:
            V.tensor_scalar(rz[:, :], nZ[:, :], 0.35, None, ALU.mult)
        else:
            V.reciprocal(rz[:, :], nZ[:, :])
        h = newt(f"h{v}")
        w_ = newt(f"w{v}")
        V.tensor_tensor(h[:, :], nY[:, :], rz[:, :], ALU.mult)
        V.tensor_scalar(h[:, :], h[:, :], 248.0, 112.0, ALU.mult, ALU.add)
        V.tensor_scalar(h[:, :], h[:, :], 0.0, 223.0, ALU.max, ALU.min)
        V.tensor_tensor(w_[:, :], X[:, :], rz[:, :], ALU.mult)
        V.tensor_scalar(w_[:, :], w_[:, :], 248.0, 112.0, ALU.mult, ALU.add)
        V.tensor_scalar(w_[:, :], w_[:, :], 0.0, 223.0, ALU.max, ALU.min)
        HV[v], WV[v] = h, w_

        fx = newt(f"fx0{v}")
        fy = newt(f"fy0{v}")
        x1 = newt("x1s")
        y1 = newt("y1s")
        xi = newt("xis", dt=I32)
        floor_pair(h, x1, fx, xi)
        floor_pair(w_, y1, fy, xi)
        FX0[v], FY0[v] = fx, fy
        # column parity: window cols = 2jp + {0,1,2,3}
        p_ = newt(f"p{v}")
        jp2 = newt("jp2s")
        V.tensor_copy(xi[:, :], y1[:, :])
        V.tensor_scalar(xi[:, :], xi[:, :], 1, None, ALU.bitwise_and)
        V.tensor_copy(p_[:, :], xi[:, :])
        PV[v] = p_
        V.tensor_tensor(jp2[:, :], y1[:, :], p_[:, :], ALU.subtract)
        V.tensor_scalar(jp2[:, :], jp2[:, :], 0.5, None, ALU.mult)
        # one merged-table index per (point, view)
        V.scalar_tensor_tensor(
            idx_i[:, v, :], x1[:, :], 112.0, jp2[:, :], ALU.mult, ALU.add
        )

    # ---- pass 2: bilinear weights (pair-duplicated fp16 tiles)
    for v in range(3):
        for l in range(3):
            Hl, Wl, Cl, Sl = LEV[l]
            # One tile per (v, l): a packed per-view tile would leave the
            # combine's wb slice with non-collapsible dims (>3D ISA limit).
            wt = wpool.tile(
                [P, m_total, 2 * Sl], F16, tag=f"wt{v}{l}", name=f"wt{v}{l}"
            )
            w_tiles[(v, l)] = wt
            if l == 0:
                fx, fy, p_ = FX0[v], FY0[v], PV[v]
            else:
                inv_s = 1.0 / SCALES[l]
                x = newt("xs")
                y = newt("ys")
                V.tensor_scalar(x[:, :], HV[v][:, :], inv_s, None, ALU.mult)
                V.tensor_scalar(y[:, :], WV[v][:, :], inv_s, None, ALU.mult)
                fx = newt("fxs")
                fy = newt("fys")
                x1 = newt("x1s")
                y1 = newt("y1s")
                xi = newt("xis", dt=I32)
                floor_pair(x, x1, fx, xi)
                floor_pair(y, y1, fy, xi)
            # wx1 = (fx>0) - fx ; wx2 = fx (same for y)
            wx1 = newt("wx1s")
            wy1 = newt("wy1s")
            V.tensor_scalar(wx1[:, :], fx[:, :], 0.0, None, ALU.is_gt)
            V.tensor_tensor(wx1[:, :], wx1[:, :], fx[:, :], ALU.subtract)
            V.tensor_scalar(wy1[:, :], fy[:, :], 0.0, None, ALU.is_gt)
            V.tensor_tensor(wy1[:, :], wy1[:, :], fy[:, :], ALU.subtract)

            if l == 0:
                # col weights: A0 = wy1*(1-p); A1 = wy1*p + wy2*(1-p); A2 = wy2*p
                t1 = newt("t1s")
                t2 = newt("t2s")
                a0 = newt("a0s")
                a1 = newt("a1s")
                V.tensor_tensor(t1[:, :], wy1[:, :], p_[:, :], ALU.mult)
                V.tensor_tensor(a0[:, :], wy1[:, :], t1[:, :], ALU.subtract)
                V.tensor_tensor(t2[:, :], fy[:, :], p_[:, :], ALU.mult)
                V.tensor_tensor(a1[:, :], t1[:, :], fy[:, :], ALU.add)
                V.tensor_tensor(a1[:, :], a1[:, :], t2[:, :], ALU.subtract)
                # slots s = r*4 + c ; rows weights (wx1, fx)
                for r, rw in ((0, wx1), (1, fx)):
                    for c, cw in ((0, a0), (1, a1), (2, t2)):
                        s = r * 4 + c
                        V.tensor_tensor(
                            wt[:, :, 2 * s : 2 * s + 2],
                            dup2(rw), dup2(cw), ALU.mult,
                        )
                # zero cols 3 and 7 (slots 3, 7)
                V.tensor_scalar(wt[:, :, 6:8], dup2(p_), 0.0, None, ALU.mult)
                V.tensor_scalar(wt[:, :, 14:16], dup2(p_), 0.0, None, ALU.mult)
            else:
                # slots s = r*2 + c: (wx1*wy1, wx1*wy2, wx2*wy1, wx2*wy2)
                V.tensor_tensor(wt[:, :, 0:2], dup2(wx1), dup2(wy1), ALU.mult)
                V.tensor_tensor(wt[:, :, 2:4], dup2(wx1), dup2(fy), ALU.mult)
                V.tensor_tensor(wt[:, :, 4:6], dup2(fx), dup2(wy1), ALU.mult)
                V.tensor_tensor(wt[:, :, 6:8], dup2(fx), dup2(fy), ALU.mult)

    # ---- per-tile: relayout idx, gather, combine, stats, store
    # dma_gather wants idx g=(m*128+p) at [g%16, g//16] in a 16-partition
    # block, replicated across the 128 partitions. With p = 32*r4+16*r2+q
    # that is wr[q, j, 8m + 2*r4 + r2] = idx[p, j, m]. Engine APs can only
    # start at partitions 0/32/64/96, so fold in two steps.
    # Prepared PREP_AHEAD tiles early so the fold->dma->gather latency is
    # hidden under earlier tiles' combine work.
    PREP_AHEAD = 2
    wrs = {}

    def prep_tile(t):
        sl = slice(t * mt, (t + 1) * mt)
        wr = ipool.tile(
            [P, 3, 8 * mt], I16, tag="wrapped", name="wrapped", bufs=4
        )
        # fuse the (r2=0, r2=1) int16 pair into one contiguous int32 write:
        # stride-2 int16 stores would trigger per-element read-modify-write.
        # A-halves (partitions 32r4+0..15) fold-cast to int32 on the idle
        # Scalar engine (values < 2^24, exact through its fp32 path);
        # B-halves (starts 16/48/80/112 are DMA-only) come via Sync.
        iA32 = ipool.tile([16, 3, mt, 4], I32, tag="iA32", name="iA32", bufs=4)
        iB16 = ipool.tile([16, 4, 3, mt], I16, tag="iB16", name="iB16", bufs=4)
        iB32 = ipool.tile([16, 4, 3, mt], I32, tag="iB32", name="iB32", bufs=4)
        for r4 in range(4):
            S.activation(
                iA32[:, :, :, r4],
                idx_i[32 * r4 : 32 * r4 + 16, :, sl],
                ACTF.Copy,
            )
            IO.dma_start(
                out=iB16[:, r4, :, :],
                in_=idx_i[32 * r4 + 16 : 32 * (r4 + 1), :, sl],
            )
        S.activation(iB32[:, :, :, :], iB16[:, :, :, :], ACTF.Copy)
        V.tensor_scalar(
            iB32[:, :, :, :], iB32[:, :, :, :], 16, None,
            ALU.logical_shift_left,
        )
        wr32 = wr[0:16, :, :].bitcast(I32)  # [16, 3, 4*mt]; word = 4m + r4
        V.tensor_tensor(
            wr32.rearrange("p j (m k) -> p j m k", k=4),
            iB32[:, :, :, :].transpose([0, 2, 3, 1]),
            iA32[:, :, :, :], ALU.bitwise_or,
        )
        IO.dma_start(out=wr[16:32, :, :], in_=wr[0:16, :, :])
        IO.dma_start(out=wr[32:64, :, :], in_=wr[0:32, :, :])
        IO.dma_start(out=wr[64:128, :, :], in_=wr[0:64, :, :])
        wrs[t] = wr

    for t in range(min(PREP_AHEAD + 1, nt)):
        prep_tile(t)

    for t in range(nt):
        sl = slice(t * mt, (t + 1) * mt)
        wr = wrs.pop(t)
        F_t = fpool.tile([P, mt, 3 * 112], F16, tag="F", name="F")
        gts = []
        for v in range(3):
            # one merged gather per (tile, view): all 3 levels in one entry
            gt = gpool.tile(
                [P, mt, ENT_ALL], F16, tag="gv", name="gv", bufs=8,
            )
            gts.append(gt)
            if SKIP_GATHER:
                V.memset(gt[:, :, :], 0)
            else:
                G.dma_gather(
                    gt[:, :, :],
                    tabs_d[v][:, :],
                    wr[:, v, :],
                    P * mt,
                    nidx_reg,
                    ENT_ALL,
                    queue_num=(v + t) % 4,
                    single_packet=False,
                )
        if t + PREP_AHEAD + 1 < nt:
            prep_tile(t + PREP_AHEAD + 1)
        for v in range(3):
            gt = gts[v]
            for l in range(3):
                Hl, Wl, Cl, Sl = LEV[l]
                eo = ENT_OFF[l]  # fp16-element offset of this level's slots
                E = Cl * Sl
                gl = gt[:, :, eo : eo + E]
                # entries are slot-major [S, C]; weights pair-duplicated.
                # Multiply in place, then sum slots by halving contiguous
                # blocks — every stage reads/writes long unit-stride runs.
                gf = gl.rearrange("p m (s c2 d) -> p m s c2 d", s=Sl, d=2)
                wb = (
                    w_tiles[(v, l)][:, sl, :]
                    .rearrange("p m (s d) -> p m s d", d=2)
                    .unsqueeze(3)
                    .broadcast_to([P, mt, Sl, Cl // 2, 2])
                )
                V.tensor_tensor(gf, gf, wb, ALU.mult)
                off = v * 112 + (0 if l == 0 else (16 if l == 1 else 48))
                Fsl = F_t[:, :, off : off + Cl]
                tr = fpool.tile([P, mt, 128], F16, tag="tr", name="tr", bufs=3)
                half = E // 2
                V.tensor_tensor(
                    tr[:, :, 0:half],
                    gl[:, :, 0:half], gl[:, :, half:E], ALU.add,
                )
                if l == 0:
                    # S=8: one extra halving (64 -> 32 -> 16)
                    V.tensor_tensor(
                        tr[:, :, 64:96],
                        tr[:, :, 0:32], tr[:, :, 32:64], ALU.add,
                    )
                    V.tensor_tensor(
                        Fsl, tr[:, :, 64:80], tr[:, :, 80:96], ALU.add
                    )
                else:
                    V.tensor_tensor(
                        Fsl, tr[:, :, 0:Cl], tr[:, :, Cl : 2 * Cl], ALU.add
                    )

        # ---- stats across views
        out_t = opool.tile([P, mt, 336], F16, tag="out", name="out_t")
        F0 = F_t[:, :, 0:112]
        F1 = F_t[:, :, 112:224]
        F2 = F_t[:, :, 224:336]
        fmax = out_t[:, :, 0:112]
        V.tensor_tensor(fmax, F0, F1, ALU.max)
        V.tensor_tensor(fmax, fmax, F2, ALU.max)
        fmean = out_t[:, :, 112:224]
        V.tensor_tensor(fmean, F0, F1, ALU.add)
        V.tensor_tensor(fmean, fmean, F2, ALU.add)
        S.activation(fmean, fmean, ACTF.Copy, scale=1.0 / 3.0)
        # std = sqrt(mean((x - mu)^2)): the centered form keeps small stds
        # accurate in fp16 (x - mu is exact for close values).
        trs = fpool.tile([P, mt, 128], F16, tag="tr", name="trs", bufs=3)
        dt_ = trs[:, :, 0:112]
        sqa = fpool.tile([P, mt, 112], F16, tag="sqa", name="sqa")
        ssq = out_t[:, :, 224:336]
        V.tensor_tensor(dt_, F0, fmean, ALU.subtract)
        S.activation(ssq, dt_, ACTF.Square)
        V.tensor_tensor(dt_, F1, fmean, ALU.subtract)
        S.activation(sqa[:, :, :], dt_, ACTF.Square)
        V.tensor_tensor(ssq, ssq, sqa[:, :, :], ALU.add)
        V.tensor_tensor(dt_, F2, fmean, ALU.subtract)
        S.activation(sqa[:, :, :], dt_, ACTF.Square)
        V.tensor_tensor(ssq, ssq, sqa[:, :, :], ALU.add)
        S.activation(out_t[:, :, 224:336], ssq, ACTF.Sqrt, scale=1.0 / 3.0)

        IO.dma_start(out=out_d[:, sl, :], in_=out_t[:, :, :])


def build_kernel(m_total=M, mt=MT, tab_rows=None):
    """Build the Bass module. Returns (nc, names) with dram tensor names."""
    if tab_rows is None:
        tab_rows = TAB_ROWS
    nc = bacc.Bacc("TRN2", num_swdge_queues=4)
    coords = nc.dram_tensor("coords", [3, P, m_total], F32, kind="ExternalInput")
    afp = nc.dram_tensor("afp", [P, 40], F32, kind="ExternalInput")
    tabs = {}
    for v in range(3):
        tabs[v] = nc.dram_tensor(
            f"tab_{v}", [tab_rows, ENT_ALL], F16, kind="ExternalInput"
        )
    out = nc.dram_tensor("out", [P, m_total, 336], F16, kind="ExternalOutput")

    with nc.allow_low_precision("fp16 sampling kernel"), TileContext(nc) as tc:
        pools_ctx = []
        import contextlib

        stack = contextlib.ExitStack()
        pools = {
            "sc": stack.enter_context(tc.tile_pool(name="sc", bufs=1)),
            "w": stack.enter_context(tc.tile_pool(name="w", bufs=1)),
            "g": stack.enter_context(tc.tile_pool(name="g", bufs=2)),
            "f": stack.enter_context(tc.tile_pool(name="f", bufs=2)),
            "o": stack.enter_context(tc.tile_pool(name="o", bufs=2)),
            "i": stack.enter_context(tc.tile_pool(name="i", bufs=2)),
        }
        dram = {
            "coords": coords.ap(),
            "afp": afp.ap(),
            "tabs": {k: v.ap() for k, v in tabs.items()},
            "out": out.ap(),
        }
        with stack:
            emit_body(nc, tc, pools, dram, m_total=m_total, mt=mt)
    nc.compile()
    return nc


# ------------------------------------------------------------------ frontend
_NC_CACHE = {}
TRACE = False
LAST_RES = [None]


def _get_nc():
    if "nc" not in _NC_CACHE:
        _NC_CACHE["nc"] = build_kernel()
    return _NC_CACHE["nc"]


def kernel(coord, img_feat0, img_feat1, img_feat2, cameras):
    coord = np.asarray(coord, np.float32)
    afp = _build_affine_plane(np.asarray(cameras, np.float32))
    tabs = _build_tables(img_feat0, img_feat1, img_feat2)

    nc = _get_nc()
    in_maps = []
    for k in range(N_CORES):
        shard = coord[k * N_CORE_PTS : (k + 1) * N_CORE_PTS]  # [32768, 3]
        cs = np.ascontiguousarray(
            shard.reshape(P, M, 3).transpose(2, 0, 1)
        )  # [3, P, M]
        im = {"coords": cs, "afp": afp}
        for v in range(3):
            im[f"tab_{v}"] = tabs[v]
        in_maps.append(im)

    res = run_bass_kernel_spmd(
        nc, in_maps, core_ids=list(range(N_CORES)), trace=TRACE
    )
    LAST_RES[0] = res
    stats = np.concatenate(
        [res.results[k]["out"].reshape(N_CORE_PTS, 336) for k in range(N_CORES)], 0
    ).astype(np.float32)
    return np.concatenate([coord, stats], axis=1)

